# revision 18
# baseline (speedup 1.0000x reference)
import os, sys
import numpy as np
import ml_dtypes

sys.path.insert(0, "/opt/trn_rl_repo")
sys.path.insert(0, "/opt/trn_rl_repo/concourse")

import concourse.bass as bass
import concourse.bacc as bacc
import concourse.tile as tile
import concourse.mybir as mybir
from concourse.bass_utils import run_bass_kernel_spmd
from concourse.masks import make_identity

BF16 = mybir.dt.bfloat16
F32 = mybir.dt.float32
I32 = mybir.dt.int32
FP8 = mybir.dt.float8e4
bf16 = ml_dtypes.bfloat16

# message/Z tables in fp8_e4m3 halve AllGather + gather traffic; numpy sim
# puts the end-to-end rel err at 1.1e-2 (gate 2e-2). K_FP8=0 falls back.
USE_FP8 = bool(int(os.environ.get("K_FP8", "1")))
ZT = FP8 if USE_FP8 else BF16

NCORES = 8
A, B, NB = 100000, 200000, 6
BOND_F, ATOM_F, H, RO = 147, 133, 300, 512
BOND_F2 = BOND_F + 1  # + ones row carrying b_i
ATOM_F2 = ATOM_F + 1  # + ones row carrying b_o
DEPTH = 6
APM = 25  # atoms per mol
BK = B // NCORES  # 25000 bonds/core
AK = A // NCORES  # 12500 atoms/core
MK = AK // APM  # 500 mols/core
BT = 196  # bond tiles of 128 (196*128 = 25088 >= 25000)
BKP = BT * 128  # padded bonds per core
AT = 100  # atom tiles of 125 (100*125 = 12500)
ZROWS = NCORES * BKP

NITERS = int(os.environ.get("K_NITERS", str(DEPTH - 1)))  # timing probe only
SKIP_S0 = bool(int(os.environ.get("K_SKIP_S0", "0")))  # timing probe only
SKIP_FINAL = bool(int(os.environ.get("K_SKIP_FINAL", "0")))  # timing probe only
SKIP_ATOM = bool(int(os.environ.get("K_SKIP_ATOM", "0")))  # timing probe only
SKIP_BT = bool(int(os.environ.get("K_SKIP_BT", "0")))  # timing probe only
UNROLL = int(os.environ.get("K_UNROLL", "28"))
NCHUNK = int(os.environ.get("K_NCHUNK", "1"))
FUNROLL = int(os.environ.get("K_FUNROLL", "20"))
TPC = BT // NCHUNK  # bond tiles per AG chunk
CR = TPC * 128  # z rows per (chunk, core)
assert BT % NCHUNK == 0 and TPC % UNROLL == 0 and AT % FUNROLL == 0

_NC_CACHE = None
SKIP_GATHERS = bool(int(os.environ.get("K_SKIP_GATHERS", "0")))
SKIP_AGS = bool(int(os.environ.get("K_SKIP_AGS", "0")))
# multi-index (fused) indirect DMA is broken on HW ucode: wrong results and
# ~90us/instruction. Single-index gathers only.
SPLIT_GATHER = True
NSWQ = int(os.environ.get("K_NSWQ", "4"))


def _gather(nc, out, src, ixcol, q):
    """Single-index indirect gather, optionally routed to SWDGE queue q."""
    inst = nc.gpsimd.indirect_dma_start(
        out=out,
        out_offset=None,
        in_=src,
        in_offset=bass.IndirectOffsetOnAxis(ap=ixcol, axis=0),
    )
    if NSWQ > 1:
        inst.ins.queue = f"qPoolDynamic{q % NSWQ or ''}"
    return inst

WCHUNKS = ((0, 128), (128, 128), (256, 44))


def _emit_bond_tail(nc, sb, ps, ident, wh_c, zslice, ti, msg_t):
    """Transpose msg tile and matmul against weight chunks; write Z rows into
    the group SBUF slice `zslice` (caller DMAs the whole group at once)."""
    if SKIP_BT:
        nc.vector.tensor_copy(zslice, msg_t[:])
        return
    zp = ps.tile([128, H], F32, name="zp", tag="zp")
    for c, (c0, cw) in enumerate(WCHUNKS):
        tp = ps.tile([128, 128], BF16, name="tp", tag="tp")
        nc.tensor.transpose(tp[:cw, :], msg_t[:, c0 : c0 + cw], ident[:])
        mT = sb.tile([128, 128], BF16, name="mT", tag="mT")
        nc.scalar.activation(mT[:cw, :], tp[:cw, :], mybir.ActivationFunctionType.Copy)
        nc.tensor.matmul(zp[:], mT[:cw, :], wh_c[c], start=(c == 0), stop=(c == 2))
    nc.scalar.activation(zslice, zp[:], mybir.ActivationFunctionType.Copy)


def _emit_ag(nc, zloc_c, zbuf, c, rg):
    if SKIP_AGS:
        nc.sync.dma_start(out=zbuf[c * NCORES * CR : c * NCORES * CR + CR, :],
                          in_=zloc_c[:])
    else:
        nc.gpsimd.collective_compute(
            "AllGather", mybir.AluOpType.bypass, replica_groups=rg,
            ins=[zloc_c[:].opt()],
            outs=[zbuf[c * NCORES * CR : (c + 1) * NCORES * CR, :].opt()],
        )


def build():
    nc = bacc.Bacc(
        "TRN2", target_bir_lowering=False, debug=False, num_devices=NCORES,
        num_swdge_queues=NSWQ,
    )

    # ---------------- inputs ----------------
    fbT = nc.dram_tensor("fbT", [BOND_F2, BT, 128], BF16, kind="ExternalInput")
    faT = nc.dram_tensor("faT", [ATOM_F2, AT, 125], BF16, kind="ExternalInput")
    # per unroll-group packed indices: [group, 128, UNROLL*2] (P row, Z rev row)
    it_idx = nc.dram_tensor("it_idx", [BT // UNROLL, 128, UNROLL * 2], I32,
                            kind="ExternalInput")
    fin_idx = nc.dram_tensor("fin_idx", [AT // FUNROLL, 125, FUNROLL * 6], I32,
                             kind="ExternalInput")
    wi = nc.dram_tensor("wi", [BOND_F2, H], BF16, kind="ExternalInput")
    wh = nc.dram_tensor("wh", [H, H], BF16, kind="ExternalInput")
    wo1 = nc.dram_tensor("wo1", [ATOM_F2, H], BF16, kind="ExternalInput")
    wo2 = nc.dram_tensor("wo2", [H, H], BF16, kind="ExternalInput")
    wfc0 = nc.dram_tensor("wfc0", [H, RO], BF16, kind="ExternalInput")
    wfc1 = nc.dram_tensor("wfc1", [RO, RO], BF16, kind="ExternalInput")
    wlast = nc.dram_tensor("wlast", [RO, 1], BF16, kind="ExternalInput")
    bh_r = nc.dram_tensor("bh_r", [128, H], F32, kind="ExternalInput")
    bfc0 = nc.dram_tensor("bfc0", [RO, 1], F32, kind="ExternalInput")
    bfc1 = nc.dram_tensor("bfc1", [RO, 1], F32, kind="ExternalInput")
    blast = nc.dram_tensor("blast", [1, 1], F32, kind="ExternalInput")
    psel_in = nc.dram_tensor("psel_in", [125, 5], BF16, kind="ExternalInput")
    out_ext = nc.dram_tensor("out", [1, MK], F32, kind="ExternalOutput")

    with tile.TileContext(nc) as tc:
        with (
            tc.tile_pool(name="const", bufs=1) as cst,
            tc.tile_pool(name="dram", bufs=1, space="DRAM") as dram,
        ):
            # big DRAM buffers
            zbufs = [
                dram.tile([ZROWS, H], ZT, addr_space="Shared", name=f"zfull{t}")
                for t in range(DEPTH)
            ]
            zlocs = [dram.tile([CR, H], ZT, name=f"zloc{c}") for c in range(NCHUNK)]
            zloc_vs = [z[:].rearrange("(t p) h -> t p h", p=128) for z in zlocs]
            # P tables: per-atom sums of Z rows (P[a] = sum_j Z[a2b[a][j]]);
            # row index == global atom id (atoms sharded contiguously, AK=12500)
            # bf16 (not fp8): P is consumed once per bond, and skipping the
            # extra fp8 rounding keeps end-to-end rel err at ~1e-2
            pbufs = [
                dram.tile([A, H], BF16, addr_space="Shared", name=f"pfull{t}")
                for t in range(DEPTH - 1)
            ]
            ploc = dram.tile([AK, H], BF16, name="ploc")
            ploc_gv = ploc[:].rearrange("(g u p) h -> g p u h", u=FUNROLL, p=125)
            # inb staging, group-fat layout: one contiguous 16.8KB run per
            # partition per group -> big DMA descriptors on load/store
            inpb_d = dram.tile([BT // UNROLL, 128, UNROLL, H], BF16, name="inpb_d")
            mv_d = dram.tile([AT, 5, H], F32, name="mv_d")

            # resident constants
            ident = cst.tile([128, 128], BF16, name="ident")
            make_identity(nc, ident[:])
            wi_c0 = cst.tile([128, H], BF16, name="wi_c0")
            wi_c1 = cst.tile([BOND_F2 - 128, H], BF16, name="wi_c1",
                             padded_shape=[128, None])
            nc.sync.dma_start(out=wi_c0[:], in_=wi[0:128, :])
            nc.sync.dma_start(out=wi_c1[:], in_=wi[128:BOND_F2, :])
            wh_c, wo2_c = [], []
            for c, (c0, cw) in enumerate(WCHUNKS):
                t1 = cst.tile([cw, H], BF16, name=f"wh_c{c}", padded_shape=[128, None])
                nc.sync.dma_start(out=t1[:], in_=wh[c0 : c0 + cw, :])
                wh_c.append(t1[:])
                t2 = cst.tile([cw, H], BF16, name=f"wo2_c{c}", padded_shape=[128, None])
                nc.sync.dma_start(out=t2[:], in_=wo2[c0 : c0 + cw, :])
                wo2_c.append(t2[:])
            wo1_c0 = cst.tile([128, H], BF16, name="wo1_c0")
            wo1_c1 = cst.tile([ATOM_F2 - 128, H], BF16, name="wo1_c1",
                              padded_shape=[128, None])
            nc.sync.dma_start(out=wo1_c0[:], in_=wo1[0:128, :])
            nc.sync.dma_start(out=wo1_c1[:], in_=wo1[128:ATOM_F2, :])
            bh_t = cst.tile([128, H], F32, name="bh_t")
            nc.sync.dma_start(out=bh_t[:], in_=bh_r[:])
            psel = cst.tile([125, 5], BF16, name="psel")
            nc.sync.dma_start(out=psel[:], in_=psel_in[:])

            rg = [list(range(NCORES))]

            # ---------------- stage 0: inp/msg0/Z0 ----------------
            assert NCHUNK == 1
            zloc_gv = zlocs[0][:].rearrange("(g u p) h -> g p u h", u=UNROLL, p=128)
            with (
                tc.tile_pool(name="s0sb", bufs=UNROLL) as sb,
                tc.tile_pool(name="s0g", bufs=2) as gsb,
                tc.tile_pool(name="s0ps", bufs=2, space="PSUM") as ps,
            ):
                for ch in range(NCHUNK if not SKIP_S0 else 0):
                    with tc.For_i(ch * TPC, (ch + 1) * TPC, UNROLL) as i0:
                        zg = gsb.tile([128, UNROLL, H], ZT, name="zg", tag="zg")
                        ig = gsb.tile([128, UNROLL, H], BF16, name="ig", tag="ig")
                        for u in range(UNROLL):
                            ti = i0 + u
                            fb0 = sb.tile([128, 128], BF16, name="fb0", tag="fb0")
                            fb1 = sb.tile(
                                [BOND_F2 - 128, 128], BF16, name="fb1", tag="fb1",
                                padded_shape=[128, None],
                            )
                            nc.sync.dma_start(out=fb0[:], in_=fbT[0:128, ti, :])
                            nc.sync.dma_start(out=fb1[:], in_=fbT[128:BOND_F2, ti, :])
                            ip = ps.tile([128, H], F32, name="ip", tag="ip")
                            nc.tensor.matmul(ip[:], fb0[:], wi_c0[:], start=True,
                                             stop=False)
                            nc.tensor.matmul(
                                ip[:], fb1[: BOND_F2 - 128, :],
                                wi_c1[: BOND_F2 - 128, :], start=False, stop=True
                            )
                            msg_t = sb.tile([128, H], BF16, name="msg_t", tag="msg_t")
                            nc.scalar.activation(
                                msg_t[:], ip[:], mybir.ActivationFunctionType.Relu
                            )
                            nc.vector.tensor_tensor(
                                out=ig[:, u, :], in0=ip[:], in1=bh_t[:],
                                op=mybir.AluOpType.add,
                            )
                            _emit_bond_tail(nc, sb, ps, ident, wh_c,
                                            zg[:, u, :], ti, msg_t)
                        nc.sync.dma_start(out=inpb_d[i0 // UNROLL], in_=ig[:])
                        nc.sync.dma_start(out=zloc_gv[i0 // UNROLL], in_=zg[:])
                    _emit_ag(nc, zlocs[ch], zbufs[0], ch, rg)

            # ---------------- iterations 1..5 ----------------
            # P[a] = sum_j Z[a2b[a][j]] on owned atoms (6 gathers / 125 atoms),
            # AllGather P, then each bond needs only 2 gathers:
            # acc[b] = inb[b] + P[b2a[b]] - Z[b2revb[b]]
            for it in range(NITERS):
                last = it == DEPTH - 2
                w_chunks = wo2_c if last else wh_c
                src = zbufs[it]
                dst = zbufs[it + 1]
                # atom stage: P_it from Z_it
                with (
                    tc.tile_pool(name=f"a{it}sb", bufs=FUNROLL) as sb,
                    tc.tile_pool(name=f"a{it}g", bufs=2) as gsb,
                ):
                    with tc.For_i(0, AT if not SKIP_ATOM else FUNROLL, FUNROLL) as fv:
                        fixg = sb.tile([125, FUNROLL * 6], I32, name="fixg",
                                       tag="fixg")
                        nc.sync.dma_start(out=fixg[:], in_=fin_idx[fv // FUNROLL])
                        pg = gsb.tile([125, FUNROLL, H], BF16, name="pg", tag="pg")
                        for u in range(FUNROLL):
                            ti = fv + u
                            g = sb.tile([125, 6, H], ZT, name="ag", tag="ag")
                            if SKIP_GATHERS:
                                nc.gpsimd.memset(g[:, 0, :], 0.01)
                            else:
                                for j in range(6):
                                    _gather(
                                        nc, g[:, j, :], src[:],
                                        fixg[:, u * 6 + j : u * 6 + j + 1], j,
                                    )
                            pacc = sb.tile([125, H], F32, name="pacc", tag="pacc")
                            nc.vector.reduce_sum(
                                pacc[:],
                                g[:].rearrange("p j h -> p h j"),
                                axis=mybir.AxisListType.X,
                            )
                            nc.scalar.activation(
                                pg[:, u, :], pacc[:],
                                mybir.ActivationFunctionType.Copy
                            )
                        nc.sync.dma_start(out=ploc_gv[fv // FUNROLL], in_=pg[:])
                    if SKIP_AGS:
                        nc.sync.dma_start(out=pbufs[it][0:AK, :], in_=ploc[:])
                    else:
                        nc.gpsimd.collective_compute(
                            "AllGather", mybir.AluOpType.bypass, replica_groups=rg,
                            ins=[ploc[:].opt()], outs=[pbufs[it][:].opt()],
                        )
                # bond stage: rev + P gathers inline, fully pipelined
                with (
                    tc.tile_pool(name=f"i{it}sb", bufs=UNROLL) as sb,
                    tc.tile_pool(name=f"i{it}g", bufs=2) as gsb,
                    tc.tile_pool(name=f"i{it}ps", bufs=4, space="PSUM") as ps,
                ):
                    for ch in range(NCHUNK):
                        with tc.For_i(ch * TPC, (ch + 1) * TPC, UNROLL) as iv:
                            ixg = sb.tile([128, UNROLL * 2], I32, name="ixg", tag="ixg")
                            nc.sync.dma_start(out=ixg[:], in_=it_idx[iv // UNROLL])
                            li = gsb.tile([128, UNROLL, H], BF16, name="li", tag="li")
                            nc.sync.dma_start(out=li[:], in_=inpb_d[iv // UNROLL])
                            zg = gsb.tile([128, UNROLL, H], ZT, name="zg", tag="zg")
                            for u in range(UNROLL):
                                ti = iv + u
                                gz = sb.tile([128, H], ZT, name="gz", tag="gz")
                                gp = sb.tile([128, H], BF16, name="gp", tag="gp")
                                if SKIP_GATHERS:
                                    nc.gpsimd.memset(gz[:], 0.01)
                                    nc.gpsimd.memset(gp[:], 0.01)
                                else:
                                    # gz deps only on Z_{t-1}: queues 0-1 so the
                                    # stream flows while AG_P is on the wire;
                                    # gp (waits AG_P) pinned to queues 2-3.
                                    _gather(nc, gz[:], src[:],
                                            ixg[:, u * 2 + 1 : u * 2 + 2], u % 2)
                                    _gather(nc, gp[:], pbufs[it][:],
                                            ixg[:, u * 2 : u * 2 + 1], 2 + u % 2)
                                acc = sb.tile([128, H], F32, name="acc", tag="acc")
                                nc.vector.tensor_tensor(
                                    out=acc[:], in0=li[:, u, :], in1=gp[:],
                                    op=mybir.AluOpType.add,
                                )
                                nc.vector.tensor_tensor(
                                    out=acc[:], in0=acc[:], in1=gz[:],
                                    op=mybir.AluOpType.subtract,
                                )
                                msg_t = sb.tile([128, H], BF16, name="msg_t",
                                                tag="msg_t")
                                nc.scalar.activation(
                                    msg_t[:], acc[:], mybir.ActivationFunctionType.Relu
                                )
                                _emit_bond_tail(nc, sb, ps, ident, w_chunks,
                                                zg[:, u, :], ti, msg_t)
                            nc.sync.dma_start(out=zloc_gv[iv // UNROLL], in_=zg[:])
                        _emit_ag(nc, zlocs[ch], dst, ch, rg)

            # ---------------- final atom stage ----------------
            zo = zbufs[DEPTH - 1]  # gathered ZO table
            with (
                tc.tile_pool(name="fsb", bufs=FUNROLL) as sb,
                tc.tile_pool(name="fps", bufs=2, space="PSUM") as ps,
            ):
                with tc.For_i(0, AT if not SKIP_FINAL else FUNROLL, FUNROLL) as fv:
                    fixg = sb.tile([125, FUNROLL * 6], I32, name="fixg", tag="fixg")
                    nc.sync.dma_start(out=fixg[:], in_=fin_idx[fv // FUNROLL])
                    for u in range(FUNROLL):
                        ti = fv + u
                        g = sb.tile([125, 6, H], ZT, name="fg", tag="fg")
                        if SKIP_GATHERS:
                            nc.gpsimd.memset(g[:, 0, :], 0.01)
                        else:
                            for j in range(6):
                                _gather(
                                    nc, g[:, j, :], zo[:],
                                    fixg[:, u * 6 + j : u * 6 + j + 1], j,
                                )
                        acc = sb.tile([125, H], F32, name="facc", tag="facc")
                        nc.vector.reduce_sum(
                            acc[:],
                            g[:].rearrange("p j h -> p h j"),
                            axis=mybir.AxisListType.X,
                        )
                        fa0 = sb.tile([128, 125], BF16, name="fa0", tag="fa0")
                        fa1 = sb.tile(
                            [ATOM_F2 - 128, 125], BF16, name="fa1", tag="fa1",
                            padded_shape=[128, None],
                        )
                        nc.sync.dma_start(out=fa0[:], in_=faT[0:128, ti, :])
                        nc.sync.dma_start(out=fa1[:], in_=faT[128:ATOM_F2, ti, :])
                        ap_ = ps.tile([125, H], F32, name="ap_", tag="ap_")
                        nc.tensor.matmul(ap_[:], fa0[:, :], wo1_c0[:], start=True,
                                         stop=False)
                        nc.tensor.matmul(ap_[:], fa1[: ATOM_F2 - 128, :],
                                         wo1_c1[: ATOM_F2 - 128, :], start=False,
                                         stop=True)
                        nc.vector.tensor_tensor(
                            out=acc[:], in0=acc[:], in1=ap_[:], op=mybir.AluOpType.add
                        )
                        ah = sb.tile([125, H], BF16, name="ah", tag="ah")
                        nc.scalar.activation(
                            ah[:], acc[:], mybir.ActivationFunctionType.Relu
                        )
                        mvp = ps.tile([5, H], F32, name="mvp", tag="mvp")
                        nc.tensor.matmul(mvp[:], psel[:], ah[:], start=True, stop=True)
                        mvs = sb.tile([5, H], F32, name="mvs", tag="mvs")
                        nc.vector.tensor_copy(mvs[:], mvp[:])
                        nc.sync.dma_start(out=mv_d[ti], in_=mvs[:])

            # ---------------- readout (static) ----------------
            with (
                tc.tile_pool(name="rsb", bufs=1) as sb,
                tc.tile_pool(name="rps", bufs=1, space="PSUM") as ps,
            ):
                # build mvT [300, 500] as 3 sbuf tiles [cw, 500], scaled by 1/APM
                mt = []
                for c, (c0, cw) in enumerate(WCHUNKS):
                    t = sb.tile([cw, MK], BF16, name=f"mt{c}", padded_shape=[128, None])
                    mt.append(t)
                for q in range(4):
                    mvq = sb.tile([125, H], F32, name=f"mvq{q}")
                    nc.sync.dma_start(
                        out=mvq[:],
                        in_=mv_d[:].rearrange("t f h -> (t f) h")[
                            q * 125 : (q + 1) * 125, :
                        ],
                    )
                    mvqb = sb.tile([125, H], BF16, name=f"mvqb{q}")
                    nc.vector.tensor_copy(mvqb[:], mvq[:])
                    for c, (c0, cw) in enumerate(WCHUNKS):
                        tp = ps.tile([128, 125], BF16, name="rtp", tag="rtp")
                        nc.tensor.transpose(
                            tp[:cw, :], mvqb[:, c0 : c0 + cw], ident[:125, :125]
                        )
                        nc.scalar.activation(
                            mt[c][:, q * 125 : (q + 1) * 125],
                            tp[:cw, :],
                            mybir.ActivationFunctionType.Copy,
                            scale=1.0 / APM,
                        )
                # h0T = relu(W_fc0^T @ mvT + b_fc0): 4 M-chunks x 3 K-chunks
                h0 = []
                for m in range(4):
                    hp = ps.tile([128, MK], F32, name="h0p", tag="h0p")
                    for c, (c0, cw) in enumerate(WCHUNKS):
                        wt = sb.tile([cw, 128], BF16, name="w0t", tag="w0t",
                                     padded_shape=[128, None])
                        nc.sync.dma_start(
                            out=wt[:], in_=wfc0[c0 : c0 + cw, m * 128 : (m + 1) * 128]
                        )
                        nc.tensor.matmul(
                            hp[:], wt[:cw, :], mt[c][:cw, :], start=(c == 0),
                            stop=(c == 2)
                        )
                    bt = sb.tile([128, 1], F32, name="b0t", tag="b0t")
                    nc.sync.dma_start(out=bt[:], in_=bfc0[m * 128 : (m + 1) * 128, :])
                    ht = sb.tile([128, MK], BF16, name=f"h0_{m}")
                    nc.scalar.activation(
                        ht[:], hp[:], mybir.ActivationFunctionType.Relu, bias=bt[:]
                    )
                    h0.append(ht)
                # h1T = relu(W_fc1^T @ h0T + b_fc1)
                h1 = []
                for m in range(4):
                    hp = ps.tile([128, MK], F32, name="h1p", tag="h1p")
                    for c in range(4):
                        wt = sb.tile([128, 128], BF16, name="w1t", tag="w1t")
                        nc.sync.dma_start(
                            out=wt[:],
                            in_=wfc1[c * 128 : (c + 1) * 128, m * 128 : (m + 1) * 128],
                        )
                        nc.tensor.matmul(
                            hp[:], wt[:], h0[c][:], start=(c == 0), stop=(c == 3)
                        )
                    bt = sb.tile([128, 1], F32, name="b1t", tag="b1t")
                    nc.sync.dma_start(out=bt[:], in_=bfc1[m * 128 : (m + 1) * 128, :])
                    ht = sb.tile([128, MK], BF16, name=f"h1_{m}")
                    nc.scalar.activation(
                        ht[:], hp[:], mybir.ActivationFunctionType.Relu, bias=bt[:]
                    )
                    h1.append(ht)
                # out = W_last^T @ h1T + b_last
                op = ps.tile([1, MK], F32, name="op", tag="op")
                for c in range(4):
                    wt = sb.tile([128, 1], BF16, name="wlt", tag="wlt")
                    nc.sync.dma_start(out=wt[:], in_=wlast[c * 128 : (c + 1) * 128, :])
                    nc.tensor.matmul(
                        op[:], wt[:], h1[c][:], start=(c == 0), stop=(c == 3)
                    )
                blt = sb.tile([1, 1], F32, name="blt")
                nc.sync.dma_start(out=blt[:], in_=blast[:])
                outs = sb.tile([1, MK], F32, name="outs")
                nc.vector.tensor_tensor(
                    out=outs[:], in0=op[:], in1=blt[:].to_broadcast([1, MK]),
                    op=mybir.AluOpType.add,
                )
                nc.sync.dma_start(out=out_ext[:], in_=outs[:])

    nc.compile()
    return nc


def _prep_inputs(inputs):
    """Host-side sharding/preprocessing. Index-only work plus dtype casts."""
    f_atoms = np.asarray(inputs["f_atoms"], np.float32)
    f_bonds = np.asarray(inputs["f_bonds"], np.float32)
    a2b = np.asarray(inputs["a2b"], np.int64)
    b2a = np.asarray(inputs["b2a"], np.int64)
    b2revb = np.asarray(inputs["b2revb"], np.int64)

    # map global bond id -> Z row (chunk-blocked: rows grouped by (chunk, core))
    def zrow(idx):
        k = idx // BK
        r = idx % BK
        c = r // CR
        return ((c * NCORES + k) * CR + (r % CR)).astype(np.int32)

    # P-table row is the global atom id; Z rev row via zrow
    it_idx_g = np.stack(
        [b2a.astype(np.int32), zrow(b2revb)], axis=1
    )  # [B, 2]
    fin_idx_g = zrow(a2b)  # [A, 6]

    w = {}
    W_i = np.asarray(inputs["W_i"], np.float32)
    b_i = np.asarray(inputs["b_i"], np.float32)
    w["wi"] = np.concatenate([W_i, b_i[None, :]], axis=0).astype(bf16)
    w["wh"] = np.asarray(inputs["W_h"], np.float32).astype(bf16)
    W_o = np.asarray(inputs["W_o"], np.float32)
    b_o = np.asarray(inputs["b_o"], np.float32)
    w["wo1"] = np.concatenate([W_o[:ATOM_F], b_o[None, :]], axis=0).astype(bf16)
    w["wo2"] = W_o[ATOM_F:].astype(bf16)
    w["wfc0"] = np.asarray(inputs["W_fc0"], np.float32).astype(bf16)
    w["wfc1"] = np.asarray(inputs["W_fc1"], np.float32).astype(bf16)
    w["wlast"] = np.asarray(inputs["W_last"], np.float32).astype(bf16)
    w["bh_r"] = np.tile(np.asarray(inputs["b_h"], np.float32)[None, :], (128, 1))
    w["bfc0"] = np.asarray(inputs["b_fc0"], np.float32).reshape(RO, 1)
    w["bfc1"] = np.asarray(inputs["b_fc1"], np.float32).reshape(RO, 1)
    w["blast"] = np.asarray(inputs["b_last"], np.float32).reshape(1, 1)
    psel = np.zeros((125, 5), np.float32)
    psel[np.arange(125), np.arange(125) // APM] = 1.0
    w["psel_in"] = psel.astype(bf16)

    ones_b = np.ones((1, BKP), np.float32)
    ones_a = np.ones((1, AK), np.float32)
    in_maps = []
    for k in range(NCORES):
        bs, be = k * BK, (k + 1) * BK
        as_, ae = k * AK, (k + 1) * AK
        fbt = np.zeros((BOND_F2, BKP), np.float32)
        fbt[:BOND_F, :BK] = f_bonds[bs:be].T
        fbt[BOND_F] = ones_b
        fat = np.concatenate([f_atoms[as_:ae].T, ones_a], axis=0)
        iti = np.zeros((BKP, 2), np.int32)
        iti[:BK] = it_idx_g[bs:be]
        # pack per unroll-group: [BT//UNROLL, 128, UNROLL*2]
        iti = (
            iti.reshape(BT // UNROLL, UNROLL, 128, 2)
            .transpose(0, 2, 1, 3)
            .reshape(BT // UNROLL, 128, UNROLL * 2)
        )
        fini = fin_idx_g[as_:ae].astype(np.int32).reshape(AT, 125, 6)
        fini = (
            fini.reshape(AT // FUNROLL, FUNROLL, 125, 6)
            .transpose(0, 2, 1, 3)
            .reshape(AT // FUNROLL, 125, FUNROLL * 6)
        )
        m = dict(w)
        m["fbT"] = fbt.astype(bf16).reshape(BOND_F2, BT, 128)
        m["faT"] = fat.astype(bf16).reshape(ATOM_F2, AT, 125)
        m["it_idx"] = np.ascontiguousarray(iti)
        m["fin_idx"] = np.ascontiguousarray(fini)
        in_maps.append(m)
    return in_maps


def kernel(**inputs) -> np.ndarray:
    global _NC_CACHE
    if _NC_CACHE is None:
        _NC_CACHE = build()
    nc = _NC_CACHE
    in_maps = _prep_inputs(inputs)
    res = run_bass_kernel_spmd(nc, in_maps, core_ids=list(range(NCORES)))
    out = np.concatenate(
        [res.results[k]["out"].reshape(-1) for k in range(NCORES)], axis=0
    )
    return out.astype(np.float32)



# revision 22
# speedup vs baseline: 1.1087x; 1.1087x over previous
import os, sys
import numpy as np
import ml_dtypes

sys.path.insert(0, "/opt/trn_rl_repo")
sys.path.insert(0, "/opt/trn_rl_repo/concourse")

import concourse.bass as bass
import concourse.bacc as bacc
import concourse.tile as tile
import concourse.mybir as mybir
from concourse.bass_utils import run_bass_kernel_spmd
from concourse.masks import make_identity

BF16 = mybir.dt.bfloat16
F32 = mybir.dt.float32
I32 = mybir.dt.int32
FP8 = mybir.dt.float8e4
bf16 = ml_dtypes.bfloat16

# message/Z tables in fp8_e4m3 halve AllGather + gather traffic; numpy sim
# puts the end-to-end rel err at 1.1e-2 (gate 2e-2). K_FP8=0 falls back.
USE_FP8 = bool(int(os.environ.get("K_FP8", "1")))
ZT = FP8 if USE_FP8 else BF16
# P table dtype: bf16 default; fp8 halves AG_P wire at some precision cost
PT = FP8 if bool(int(os.environ.get("K_PFP8", "0"))) else BF16

NCORES = 8
A, B, NB = 100000, 200000, 6
BOND_F, ATOM_F, H, RO = 147, 133, 300, 512
BOND_F2 = BOND_F + 1  # + ones row carrying b_i
ATOM_F2 = ATOM_F + 1  # + ones row carrying b_o
DEPTH = 6
APM = 25  # atoms per mol
BK = B // NCORES  # 25000 bonds/core
AK = A // NCORES  # 12500 atoms/core
MK = AK // APM  # 500 mols/core
BT = 196  # bond tiles of 128 (196*128 = 25088 >= 25000)
BKP = BT * 128  # padded bonds per core
AT = 100  # atom tiles of 125 (100*125 = 12500)
ZROWS = NCORES * BKP

NITERS = int(os.environ.get("K_NITERS", str(DEPTH - 1)))  # timing probe only
SKIP_S0 = bool(int(os.environ.get("K_SKIP_S0", "0")))  # timing probe only
SKIP_FINAL = bool(int(os.environ.get("K_SKIP_FINAL", "0")))  # timing probe only
SKIP_ATOM = bool(int(os.environ.get("K_SKIP_ATOM", "0")))  # timing probe only
SKIP_BT = bool(int(os.environ.get("K_SKIP_BT", "0")))  # timing probe only
UNROLL = int(os.environ.get("K_UNROLL", "28"))
NCHUNK = int(os.environ.get("K_NCHUNK", "1"))
FUNROLL = int(os.environ.get("K_FUNROLL", "20"))
TPC = BT // NCHUNK  # bond tiles per AG chunk
CR = TPC * 128  # z rows per (chunk, core)
assert BT % NCHUNK == 0 and TPC % UNROLL == 0 and AT % FUNROLL == 0

_NC_CACHE = None
SKIP_GATHERS = bool(int(os.environ.get("K_SKIP_GATHERS", "0")))
SKIP_AGS = bool(int(os.environ.get("K_SKIP_AGS", "0")))
# multi-index (fused) indirect DMA is broken on HW ucode: wrong results and
# ~90us/instruction. Single-index gathers only.
SPLIT_GATHER = True
NSWQ = int(os.environ.get("K_NSWQ", "4"))


def _gather(nc, out, src, ixcol, q):
    """Single-index indirect gather, optionally routed to SWDGE queue q."""
    inst = nc.gpsimd.indirect_dma_start(
        out=out,
        out_offset=None,
        in_=src,
        in_offset=bass.IndirectOffsetOnAxis(ap=ixcol, axis=0),
    )
    if NSWQ > 1:
        inst.ins.queue = f"qPoolDynamic{q % NSWQ or ''}"
    return inst

WCHUNKS = ((0, 128), (128, 128), (256, 44))


def _emit_bond_tail(nc, sb, ps, ident, wh_c, zslice, ti, msg_t):
    """Transpose msg tile and matmul against weight chunks; write Z rows into
    the group SBUF slice `zslice` (caller DMAs the whole group at once)."""
    if SKIP_BT:
        nc.vector.tensor_copy(zslice, msg_t[:])
        return
    zp = ps.tile([128, H], F32, name="zp", tag="zp")
    for c, (c0, cw) in enumerate(WCHUNKS):
        tp = ps.tile([128, 128], BF16, name="tp", tag="tp")
        nc.tensor.transpose(tp[:cw, :], msg_t[:, c0 : c0 + cw], ident[:])
        mT = sb.tile([128, 128], BF16, name="mT", tag="mT")
        nc.scalar.activation(mT[:cw, :], tp[:cw, :], mybir.ActivationFunctionType.Copy)
        nc.tensor.matmul(zp[:], mT[:cw, :], wh_c[c], start=(c == 0), stop=(c == 2))
    nc.scalar.activation(zslice, zp[:], mybir.ActivationFunctionType.Copy)


def _emit_ag(nc, zloc_c, zbuf, c, rg):
    if SKIP_AGS:
        nc.sync.dma_start(out=zbuf[c * NCORES * CR : c * NCORES * CR + CR, :],
                          in_=zloc_c[:])
    else:
        nc.gpsimd.collective_compute(
            "AllGather", mybir.AluOpType.bypass, replica_groups=rg,
            ins=[zloc_c[:].opt()],
            outs=[zbuf[c * NCORES * CR : (c + 1) * NCORES * CR, :].opt()],
        )


def build():
    nc = bacc.Bacc(
        "TRN2", target_bir_lowering=False, debug=False, num_devices=NCORES,
        num_swdge_queues=NSWQ,
    )

    # ---------------- inputs ----------------
    fbT = nc.dram_tensor("fbT", [BOND_F2, BT, 128], BF16, kind="ExternalInput")
    faT = nc.dram_tensor("faT", [ATOM_F2, AT, 125], BF16, kind="ExternalInput")
    # per unroll-group packed indices: [group, 128, UNROLL*2] (P row, Z rev row)
    it_idx = nc.dram_tensor("it_idx", [BT // UNROLL, 128, UNROLL * 2], I32,
                            kind="ExternalInput")
    fin_idx = nc.dram_tensor("fin_idx", [AT // FUNROLL, 125, FUNROLL * 6], I32,
                             kind="ExternalInput")
    wi = nc.dram_tensor("wi", [BOND_F2, H], BF16, kind="ExternalInput")
    wh = nc.dram_tensor("wh", [H, H], BF16, kind="ExternalInput")
    wo1 = nc.dram_tensor("wo1", [ATOM_F2, H], BF16, kind="ExternalInput")
    wo2 = nc.dram_tensor("wo2", [H, H], BF16, kind="ExternalInput")
    wfc0 = nc.dram_tensor("wfc0", [H, RO], BF16, kind="ExternalInput")
    wfc1 = nc.dram_tensor("wfc1", [RO, RO], BF16, kind="ExternalInput")
    wlast = nc.dram_tensor("wlast", [RO, 1], BF16, kind="ExternalInput")
    bh_r = nc.dram_tensor("bh_r", [128, H], F32, kind="ExternalInput")
    bfc0 = nc.dram_tensor("bfc0", [RO, 1], F32, kind="ExternalInput")
    bfc1 = nc.dram_tensor("bfc1", [RO, 1], F32, kind="ExternalInput")
    blast = nc.dram_tensor("blast", [1, 1], F32, kind="ExternalInput")
    psel_in = nc.dram_tensor("psel_in", [125, 5], BF16, kind="ExternalInput")
    out_ext = nc.dram_tensor("out", [1, MK], F32, kind="ExternalOutput")

    with tile.TileContext(nc) as tc:
        with (
            tc.tile_pool(name="const", bufs=1) as cst,
            tc.tile_pool(name="dram", bufs=1, space="DRAM") as dram,
        ):
            # big DRAM buffers
            zbufs = [
                dram.tile([ZROWS, H], ZT, addr_space="Shared", name=f"zfull{t}")
                for t in range(DEPTH)
            ]
            zlocs = [dram.tile([CR, H], ZT, name=f"zloc{c}") for c in range(NCHUNK)]
            zloc_vs = [z[:].rearrange("(t p) h -> t p h", p=128) for z in zlocs]
            # P tables: per-atom sums of Z rows (P[a] = sum_j Z[a2b[a][j]]);
            # row index == global atom id (atoms sharded contiguously, AK=12500)
            # bf16 (not fp8): P is consumed once per bond, and skipping the
            # extra fp8 rounding keeps end-to-end rel err at ~1e-2
            pbufs = [
                dram.tile([A, H], PT, addr_space="Shared", name=f"pfull{t}")
                for t in range(DEPTH - 1)
            ]
            ploc = dram.tile([AK, H], PT, name="ploc")
            ploc_gv = ploc[:].rearrange("(g u p) h -> g p u h", u=FUNROLL, p=125)
            # inb staging, group-fat layout: one contiguous 16.8KB run per
            # partition per group -> big DMA descriptors on load/store
            inpb_d = dram.tile([BT // UNROLL, 128, UNROLL, H], BF16, name="inpb_d")
            mv_d = dram.tile([AT, 5, H], F32, name="mv_d")

            # resident constants
            ident = cst.tile([128, 128], BF16, name="ident")
            make_identity(nc, ident[:])
            wi_c0 = cst.tile([128, H], BF16, name="wi_c0")
            wi_c1 = cst.tile([BOND_F2 - 128, H], BF16, name="wi_c1",
                             padded_shape=[128, None])
            nc.sync.dma_start(out=wi_c0[:], in_=wi[0:128, :])
            nc.sync.dma_start(out=wi_c1[:], in_=wi[128:BOND_F2, :])
            wh_c, wo2_c = [], []
            for c, (c0, cw) in enumerate(WCHUNKS):
                t1 = cst.tile([cw, H], BF16, name=f"wh_c{c}", padded_shape=[128, None])
                nc.sync.dma_start(out=t1[:], in_=wh[c0 : c0 + cw, :])
                wh_c.append(t1[:])
                t2 = cst.tile([cw, H], BF16, name=f"wo2_c{c}", padded_shape=[128, None])
                nc.sync.dma_start(out=t2[:], in_=wo2[c0 : c0 + cw, :])
                wo2_c.append(t2[:])
            wo1_c0 = cst.tile([128, H], BF16, name="wo1_c0")
            wo1_c1 = cst.tile([ATOM_F2 - 128, H], BF16, name="wo1_c1",
                              padded_shape=[128, None])
            nc.sync.dma_start(out=wo1_c0[:], in_=wo1[0:128, :])
            nc.sync.dma_start(out=wo1_c1[:], in_=wo1[128:ATOM_F2, :])
            bh_t = cst.tile([128, H], F32, name="bh_t")
            nc.sync.dma_start(out=bh_t[:], in_=bh_r[:])
            psel = cst.tile([125, 5], BF16, name="psel")
            nc.sync.dma_start(out=psel[:], in_=psel_in[:])

            rg = [list(range(NCORES))]

            # ---------------- stage 0: inp/msg0/Z0 ----------------
            assert NCHUNK == 1
            zloc_gv = zlocs[0][:].rearrange("(g u p) h -> g p u h", u=UNROLL, p=128)
            with (
                tc.tile_pool(name="s0sb", bufs=UNROLL) as sb,
                tc.tile_pool(name="s0g", bufs=2) as gsb,
                tc.tile_pool(name="s0ps", bufs=2, space="PSUM") as ps,
            ):
                for ch in range(NCHUNK if not SKIP_S0 else 0):
                    with tc.For_i(ch * TPC, (ch + 1) * TPC, UNROLL) as i0:
                        zg = gsb.tile([128, UNROLL, H], ZT, name="zg", tag="zg")
                        ig = gsb.tile([128, UNROLL, H], BF16, name="ig", tag="ig")
                        for u in range(UNROLL):
                            ti = i0 + u
                            fb0 = sb.tile([128, 128], BF16, name="fb0", tag="fb0")
                            fb1 = sb.tile(
                                [BOND_F2 - 128, 128], BF16, name="fb1", tag="fb1",
                                padded_shape=[128, None],
                            )
                            nc.sync.dma_start(out=fb0[:], in_=fbT[0:128, ti, :])
                            nc.sync.dma_start(out=fb1[:], in_=fbT[128:BOND_F2, ti, :])
                            ip = ps.tile([128, H], F32, name="ip", tag="ip")
                            nc.tensor.matmul(ip[:], fb0[:], wi_c0[:], start=True,
                                             stop=False)
                            nc.tensor.matmul(
                                ip[:], fb1[: BOND_F2 - 128, :],
                                wi_c1[: BOND_F2 - 128, :], start=False, stop=True
                            )
                            msg_t = sb.tile([128, H], BF16, name="msg_t", tag="msg_t")
                            nc.scalar.activation(
                                msg_t[:], ip[:], mybir.ActivationFunctionType.Relu
                            )
                            nc.vector.tensor_tensor(
                                out=ig[:, u, :], in0=ip[:], in1=bh_t[:],
                                op=mybir.AluOpType.add,
                            )
                            nc.sync.dma_start(out=inpb_d[i0 // UNROLL, :, u, :],
                                              in_=ig[:, u, :])
                            _emit_bond_tail(nc, sb, ps, ident, wh_c,
                                            zg[:, u, :], ti, msg_t)
                            nc.sync.dma_start(out=zloc_gv[i0 // UNROLL, :, u, :],
                                              in_=zg[:, u, :])
                    _emit_ag(nc, zlocs[ch], zbufs[0], ch, rg)

            # ---------------- iterations 1..5 ----------------
            # P[a] = sum_j Z[a2b[a][j]] on owned atoms (6 gathers / 125 atoms),
            # AllGather P, then each bond needs only 2 gathers:
            # acc[b] = inb[b] + P[b2a[b]] - Z[b2revb[b]]
            for it in range(NITERS):
                last = it == DEPTH - 2
                w_chunks = wo2_c if last else wh_c
                src = zbufs[it]
                dst = zbufs[it + 1]
                # atom stage: P_it from Z_it
                with (
                    tc.tile_pool(name=f"a{it}sb", bufs=FUNROLL) as sb,
                    tc.tile_pool(name=f"a{it}g", bufs=2) as gsb,
                ):
                    with tc.For_i(0, AT if not SKIP_ATOM else FUNROLL, FUNROLL) as fv:
                        fixg = sb.tile([125, FUNROLL * 6], I32, name="fixg",
                                       tag="fixg")
                        nc.sync.dma_start(out=fixg[:], in_=fin_idx[fv // FUNROLL])
                        pg = gsb.tile([125, FUNROLL, H], PT, name="pg", tag="pg")
                        for u in range(FUNROLL):
                            ti = fv + u
                            g = sb.tile([125, 6, H], ZT, name="ag", tag="ag")
                            if SKIP_GATHERS:
                                nc.gpsimd.memset(g[:, 0, :], 0.01)
                            else:
                                for j in range(6):
                                    _gather(
                                        nc, g[:, j, :], src[:],
                                        fixg[:, u * 6 + j : u * 6 + j + 1],
                                        u * 6 + j,
                                    )
                            pacc = sb.tile([125, H], F32, name="pacc", tag="pacc")
                            nc.vector.reduce_sum(
                                pacc[:],
                                g[:].rearrange("p j h -> p h j"),
                                axis=mybir.AxisListType.X,
                            )
                            nc.scalar.activation(
                                pg[:, u, :], pacc[:],
                                mybir.ActivationFunctionType.Copy
                            )
                            nc.sync.dma_start(out=ploc_gv[fv // FUNROLL, :, u, :],
                                              in_=pg[:, u, :])
                    if SKIP_AGS:
                        nc.sync.dma_start(out=pbufs[it][0:AK, :], in_=ploc[:])
                    else:
                        nc.gpsimd.collective_compute(
                            "AllGather", mybir.AluOpType.bypass, replica_groups=rg,
                            ins=[ploc[:].opt()], outs=[pbufs[it][:].opt()],
                        )
                # bond stage: rev + P gathers inline, fully pipelined
                with (
                    tc.tile_pool(name=f"i{it}sb", bufs=UNROLL) as sb,
                    tc.tile_pool(name=f"i{it}g", bufs=2) as gsb,
                    tc.tile_pool(name=f"i{it}ps", bufs=4, space="PSUM") as ps,
                ):
                    for ch in range(NCHUNK):
                        with tc.For_i(ch * TPC, (ch + 1) * TPC, UNROLL) as iv:
                            ixg = sb.tile([128, UNROLL * 2], I32, name="ixg", tag="ixg")
                            nc.sync.dma_start(out=ixg[:], in_=it_idx[iv // UNROLL])
                            li = gsb.tile([128, UNROLL, H], BF16, name="li", tag="li")
                            nc.sync.dma_start(out=li[:], in_=inpb_d[iv // UNROLL])
                            zg = gsb.tile([128, UNROLL, H], ZT, name="zg", tag="zg")
                            for u in range(UNROLL):
                                ti = iv + u
                                gz = sb.tile([128, H], ZT, name="gz", tag="gz")
                                gp = sb.tile([128, H], PT, name="gp", tag="gp")
                                if SKIP_GATHERS:
                                    nc.gpsimd.memset(gz[:], 0.01)
                                    nc.gpsimd.memset(gp[:], 0.01)
                                else:
                                    # gz deps only on Z_{t-1}: queues 0-1 so the
                                    # stream flows while AG_P is on the wire;
                                    # gp (waits AG_P) pinned to queues 2-3.
                                    _gather(nc, gz[:], src[:],
                                            ixg[:, u * 2 + 1 : u * 2 + 2], u % 2)
                                    _gather(nc, gp[:], pbufs[it][:],
                                            ixg[:, u * 2 : u * 2 + 1], 2 + u % 2)
                                acc = sb.tile([128, H], F32, name="acc", tag="acc")
                                nc.vector.tensor_tensor(
                                    out=acc[:], in0=li[:, u, :], in1=gp[:],
                                    op=mybir.AluOpType.add,
                                )
                                nc.vector.tensor_tensor(
                                    out=acc[:], in0=acc[:], in1=gz[:],
                                    op=mybir.AluOpType.subtract,
                                )
                                msg_t = sb.tile([128, H], BF16, name="msg_t",
                                                tag="msg_t")
                                nc.scalar.activation(
                                    msg_t[:], acc[:], mybir.ActivationFunctionType.Relu
                                )
                                _emit_bond_tail(nc, sb, ps, ident, w_chunks,
                                                zg[:, u, :], ti, msg_t)
                                nc.sync.dma_start(
                                    out=zloc_gv[iv // UNROLL, :, u, :],
                                    in_=zg[:, u, :])
                        _emit_ag(nc, zlocs[ch], dst, ch, rg)

            # ---------------- final atom stage ----------------
            zo = zbufs[DEPTH - 1]  # gathered ZO table
            with (
                tc.tile_pool(name="fsb", bufs=FUNROLL) as sb,
                tc.tile_pool(name="fps", bufs=2, space="PSUM") as ps,
            ):
                with tc.For_i(0, AT if not SKIP_FINAL else FUNROLL, FUNROLL) as fv:
                    fixg = sb.tile([125, FUNROLL * 6], I32, name="fixg", tag="fixg")
                    nc.sync.dma_start(out=fixg[:], in_=fin_idx[fv // FUNROLL])
                    for u in range(FUNROLL):
                        ti = fv + u
                        g = sb.tile([125, 6, H], ZT, name="fg", tag="fg")
                        if SKIP_GATHERS:
                            nc.gpsimd.memset(g[:, 0, :], 0.01)
                        else:
                            for j in range(6):
                                _gather(
                                    nc, g[:, j, :], zo[:],
                                    fixg[:, u * 6 + j : u * 6 + j + 1],
                                    u * 6 + j,
                                )
                        acc = sb.tile([125, H], F32, name="facc", tag="facc")
                        nc.vector.reduce_sum(
                            acc[:],
                            g[:].rearrange("p j h -> p h j"),
                            axis=mybir.AxisListType.X,
                        )
                        fa0 = sb.tile([128, 125], BF16, name="fa0", tag="fa0")
                        fa1 = sb.tile(
                            [ATOM_F2 - 128, 125], BF16, name="fa1", tag="fa1",
                            padded_shape=[128, None],
                        )
                        nc.sync.dma_start(out=fa0[:], in_=faT[0:128, ti, :])
                        nc.sync.dma_start(out=fa1[:], in_=faT[128:ATOM_F2, ti, :])
                        ap_ = ps.tile([125, H], F32, name="ap_", tag="ap_")
                        nc.tensor.matmul(ap_[:], fa0[:, :], wo1_c0[:], start=True,
                                         stop=False)
                        nc.tensor.matmul(ap_[:], fa1[: ATOM_F2 - 128, :],
                                         wo1_c1[: ATOM_F2 - 128, :], start=False,
                                         stop=True)
                        nc.vector.tensor_tensor(
                            out=acc[:], in0=acc[:], in1=ap_[:], op=mybir.AluOpType.add
                        )
                        ah = sb.tile([125, H], BF16, name="ah", tag="ah")
                        nc.scalar.activation(
                            ah[:], acc[:], mybir.ActivationFunctionType.Relu
                        )
                        mvp = ps.tile([5, H], F32, name="mvp", tag="mvp")
                        nc.tensor.matmul(mvp[:], psel[:], ah[:], start=True, stop=True)
                        mvs = sb.tile([5, H], F32, name="mvs", tag="mvs")
                        nc.vector.tensor_copy(mvs[:], mvp[:])
                        nc.sync.dma_start(out=mv_d[ti], in_=mvs[:])

            # ---------------- readout (static) ----------------
            with (
                tc.tile_pool(name="rsb", bufs=1) as sb,
                tc.tile_pool(name="rps", bufs=1, space="PSUM") as ps,
            ):
                # build mvT [300, 500] as 3 sbuf tiles [cw, 500], scaled by 1/APM
                mt = []
                for c, (c0, cw) in enumerate(WCHUNKS):
                    t = sb.tile([cw, MK], BF16, name=f"mt{c}", padded_shape=[128, None])
                    mt.append(t)
                for q in range(4):
                    mvq = sb.tile([125, H], F32, name=f"mvq{q}")
                    nc.sync.dma_start(
                        out=mvq[:],
                        in_=mv_d[:].rearrange("t f h -> (t f) h")[
                            q * 125 : (q + 1) * 125, :
                        ],
                    )
                    mvqb = sb.tile([125, H], BF16, name=f"mvqb{q}")
                    nc.vector.tensor_copy(mvqb[:], mvq[:])
                    for c, (c0, cw) in enumerate(WCHUNKS):
                        tp = ps.tile([128, 125], BF16, name="rtp", tag="rtp")
                        nc.tensor.transpose(
                            tp[:cw, :], mvqb[:, c0 : c0 + cw], ident[:125, :125]
                        )
                        nc.scalar.activation(
                            mt[c][:, q * 125 : (q + 1) * 125],
                            tp[:cw, :],
                            mybir.ActivationFunctionType.Copy,
                            scale=1.0 / APM,
                        )
                # h0T = relu(W_fc0^T @ mvT + b_fc0): 4 M-chunks x 3 K-chunks
                h0 = []
                for m in range(4):
                    hp = ps.tile([128, MK], F32, name="h0p", tag="h0p")
                    for c, (c0, cw) in enumerate(WCHUNKS):
                        wt = sb.tile([cw, 128], BF16, name="w0t", tag="w0t",
                                     padded_shape=[128, None])
                        nc.sync.dma_start(
                            out=wt[:], in_=wfc0[c0 : c0 + cw, m * 128 : (m + 1) * 128]
                        )
                        nc.tensor.matmul(
                            hp[:], wt[:cw, :], mt[c][:cw, :], start=(c == 0),
                            stop=(c == 2)
                        )
                    bt = sb.tile([128, 1], F32, name="b0t", tag="b0t")
                    nc.sync.dma_start(out=bt[:], in_=bfc0[m * 128 : (m + 1) * 128, :])
                    ht = sb.tile([128, MK], BF16, name=f"h0_{m}")
                    nc.scalar.activation(
                        ht[:], hp[:], mybir.ActivationFunctionType.Relu, bias=bt[:]
                    )
                    h0.append(ht)
                # h1T = relu(W_fc1^T @ h0T + b_fc1)
                h1 = []
                for m in range(4):
                    hp = ps.tile([128, MK], F32, name="h1p", tag="h1p")
                    for c in range(4):
                        wt = sb.tile([128, 128], BF16, name="w1t", tag="w1t")
                        nc.sync.dma_start(
                            out=wt[:],
                            in_=wfc1[c * 128 : (c + 1) * 128, m * 128 : (m + 1) * 128],
                        )
                        nc.tensor.matmul(
                            hp[:], wt[:], h0[c][:], start=(c == 0), stop=(c == 3)
                        )
                    bt = sb.tile([128, 1], F32, name="b1t", tag="b1t")
                    nc.sync.dma_start(out=bt[:], in_=bfc1[m * 128 : (m + 1) * 128, :])
                    ht = sb.tile([128, MK], BF16, name=f"h1_{m}")
                    nc.scalar.activation(
                        ht[:], hp[:], mybir.ActivationFunctionType.Relu, bias=bt[:]
                    )
                    h1.append(ht)
                # out = W_last^T @ h1T + b_last
                op = ps.tile([1, MK], F32, name="op", tag="op")
                for c in range(4):
                    wt = sb.tile([128, 1], BF16, name="wlt", tag="wlt")
                    nc.sync.dma_start(out=wt[:], in_=wlast[c * 128 : (c + 1) * 128, :])
                    nc.tensor.matmul(
                        op[:], wt[:], h1[c][:], start=(c == 0), stop=(c == 3)
                    )
                blt = sb.tile([1, 1], F32, name="blt")
                nc.sync.dma_start(out=blt[:], in_=blast[:])
                outs = sb.tile([1, MK], F32, name="outs")
                nc.vector.tensor_tensor(
                    out=outs[:], in0=op[:], in1=blt[:].to_broadcast([1, MK]),
                    op=mybir.AluOpType.add,
                )
                nc.sync.dma_start(out=out_ext[:], in_=outs[:])

    nc.compile()
    return nc


def _prep_inputs(inputs):
    """Host-side sharding/preprocessing. Index-only work plus dtype casts."""
    f_atoms = np.asarray(inputs["f_atoms"], np.float32)
    f_bonds = np.asarray(inputs["f_bonds"], np.float32)
    a2b = np.asarray(inputs["a2b"], np.int64)
    b2a = np.asarray(inputs["b2a"], np.int64)
    b2revb = np.asarray(inputs["b2revb"], np.int64)

    # map global bond id -> Z row (chunk-blocked: rows grouped by (chunk, core))
    def zrow(idx):
        k = idx // BK
        r = idx % BK
        c = r // CR
        return ((c * NCORES + k) * CR + (r % CR)).astype(np.int32)

    # P-table row is the global atom id; Z rev row via zrow
    it_idx_g = np.stack(
        [b2a.astype(np.int32), zrow(b2revb)], axis=1
    )  # [B, 2]
    fin_idx_g = zrow(a2b)  # [A, 6]

    w = {}
    W_i = np.asarray(inputs["W_i"], np.float32)
    b_i = np.asarray(inputs["b_i"], np.float32)
    w["wi"] = np.concatenate([W_i, b_i[None, :]], axis=0).astype(bf16)
    w["wh"] = np.asarray(inputs["W_h"], np.float32).astype(bf16)
    W_o = np.asarray(inputs["W_o"], np.float32)
    b_o = np.asarray(inputs["b_o"], np.float32)
    w["wo1"] = np.concatenate([W_o[:ATOM_F], b_o[None, :]], axis=0).astype(bf16)
    w["wo2"] = W_o[ATOM_F:].astype(bf16)
    w["wfc0"] = np.asarray(inputs["W_fc0"], np.float32).astype(bf16)
    w["wfc1"] = np.asarray(inputs["W_fc1"], np.float32).astype(bf16)
    w["wlast"] = np.asarray(inputs["W_last"], np.float32).astype(bf16)
    w["bh_r"] = np.tile(np.asarray(inputs["b_h"], np.float32)[None, :], (128, 1))
    w["bfc0"] = np.asarray(inputs["b_fc0"], np.float32).reshape(RO, 1)
    w["bfc1"] = np.asarray(inputs["b_fc1"], np.float32).reshape(RO, 1)
    w["blast"] = np.asarray(inputs["b_last"], np.float32).reshape(1, 1)
    psel = np.zeros((125, 5), np.float32)
    psel[np.arange(125), np.arange(125) // APM] = 1.0
    w["psel_in"] = psel.astype(bf16)

    ones_b = np.ones((1, BKP), np.float32)
    ones_a = np.ones((1, AK), np.float32)
    in_maps = []
    for k in range(NCORES):
        bs, be = k * BK, (k + 1) * BK
        as_, ae = k * AK, (k + 1) * AK
        fbt = np.zeros((BOND_F2, BKP), np.float32)
        fbt[:BOND_F, :BK] = f_bonds[bs:be].T
        fbt[BOND_F] = ones_b
        fat = np.concatenate([f_atoms[as_:ae].T, ones_a], axis=0)
        iti = np.zeros((BKP, 2), np.int32)
        iti[:BK] = it_idx_g[bs:be]
        # pack per unroll-group: [BT//UNROLL, 128, UNROLL*2]
        iti = (
            iti.reshape(BT // UNROLL, UNROLL, 128, 2)
            .transpose(0, 2, 1, 3)
            .reshape(BT // UNROLL, 128, UNROLL * 2)
        )
        fini = fin_idx_g[as_:ae].astype(np.int32).reshape(AT, 125, 6)
        fini = (
            fini.reshape(AT // FUNROLL, FUNROLL, 125, 6)
            .transpose(0, 2, 1, 3)
            .reshape(AT // FUNROLL, 125, FUNROLL * 6)
        )
        m = dict(w)
        m["fbT"] = fbt.astype(bf16).reshape(BOND_F2, BT, 128)
        m["faT"] = fat.astype(bf16).reshape(ATOM_F2, AT, 125)
        m["it_idx"] = np.ascontiguousarray(iti)
        m["fin_idx"] = np.ascontiguousarray(fini)
        in_maps.append(m)
    return in_maps


def kernel(**inputs) -> np.ndarray:
    global _NC_CACHE
    if _NC_CACHE is None:
        _NC_CACHE = build()
    nc = _NC_CACHE
    in_maps = _prep_inputs(inputs)
    res = run_bass_kernel_spmd(nc, in_maps, core_ids=list(range(NCORES)))
    out = np.concatenate(
        [res.results[k]["out"].reshape(-1) for k in range(NCORES)], axis=0
    )
    return out.astype(np.float32)

